# revision 13
# baseline (speedup 1.0000x reference)
"""Single-head causal attention (B=4, T=2048, C=1024, H=64) on 8 NeuronCores.

Sharding: 8 cores = 4 batches x 2 "interleaved halves". Core (b, h) computes
query blocks of 512 rows: h=0 -> rows [0:512] and [1024:1536]; h=1 -> rows
[512:1024] and [1536:2048]. This balances causal work while keeping ONE SPMD
program: all per-core differences enter through input DATA:
  - xq:   x[b].T columns of the core's query rows          [C, 1024]
  - xk:   x[b].T columns [0:1536] (k/v prefix, shared)      [C, 1536]
  - ebias: per (block, chunk) exp bias 0 / -1e30 that kills acausal chunks
           inside the ACT exp instruction (free masking).
Compile-time schedule processes max(chunks-over-cores) per block; invalid
chunks produce exp(.. - 1e30) = 0 so they contribute nothing to either the
softmax numerator or denominator.

Layout: scores are computed transposed (scoresT[tk, tq]) so softmax sums come
from the PV matmul itself: V is augmented with a ones column -> PV psum row 64
is the denominator. Diagonal (partially-causal) chunks are masked with slices
of one shared staircase tile on DVE.
"""

import numpy as np

import concourse.bass as bass
from concourse import bacc
import concourse.mybir as mybir
import concourse.tile as tile
from concourse.bass_utils import run_bass_kernel_spmd

B, T, C, H = 4, 2048, 1024, 64
P = 128
TQ = 512                 # query block width
NBLK = 2                 # query blocks per core
NQ = NBLK * TQ           # 1024 query rows per core
SCHED = (4, 12)          # full-phase k-chunks per block (compile-time max)
NDIAG = TQ // P          # 4 diagonal chunks per block
KFULL = SCHED[-1] * P    # 1536 k columns needed for full phase
CCH = C // P             # 8 contraction chunks
SCALE = float(C) ** -0.5
NEG = -1e30

F32 = mybir.dt.float32
F32R = mybir.dt.float32r

_CACHE = {}


def _mm(ap, fast):
    # view a float32 AP as float32r for 1-cycle/row matmuls
    return ap.bitcast(F32R) if fast else ap


def build(fast_mm=True):
    DTM = F32R if fast_mm else F32   # dtype of everything feeding a matmul
    nc = bacc.Bacc()
    xq_d = nc.declare_dram_parameter("xq", [C, NQ], DTM, isOutput=False)
    xk_d = nc.declare_dram_parameter("xk", [C, KFULL], DTM, isOutput=False)
    wkv_d = nc.declare_dram_parameter("wkv", [C, 2 * H], DTM, isOutput=False)
    wqv_d = nc.declare_dram_parameter("wqv", [C, 2 * H], DTM, isOutput=False)
    wk_d = nc.declare_dram_parameter("wk", [C, H], DTM, isOutput=False)
    eb_d = nc.declare_dram_parameter("ebias", [P, NBLK * SCHED[-1]], F32, isOutput=False)
    st_d = nc.declare_dram_parameter("stair", [P, 896], F32, isOutput=False)
    id_d = nc.declare_dram_parameter("ident", [P, P], DTM, isOutput=False)
    on_d = nc.declare_dram_parameter("vones", [P, NDIAG * NBLK + SCHED[-1]], DTM, isOutput=False)
    out_d = nc.declare_dram_parameter("out", [NQ, H], F32, isOutput=True)

    NV = NDIAG * NBLK + SCHED[-1]   # 8 diag + 12 full v blocks of 128 rows
    EXPF = mybir.ActivationFunctionType.Exp

    with tile.TileContext(nc) as tc:
        with (
            tc.tile_pool(name="big", bufs=1) as big,
            tc.tile_pool(name="work", bufs=4) as work,
            tc.tile_pool(name="ps", bufs=2, space="PSUM") as psp,
            tc.tile_pool(name="ps_s", bufs=2, space="PSUM") as pss,
            tc.tile_pool(name="ps_pv", bufs=2, space="PSUM") as pspv,
            tc.tile_pool(name="ps_tr", bufs=2, space="PSUM") as pstr,
        ):
            # ---- constants / weights ----
            wkv = big.tile([P, CCH, 2 * H], DTM)
            nc.sync.dma_start(out=wkv[:], in_=wkv_d[:].rearrange("(nc p) h -> p nc h", p=P))
            wqv = big.tile([P, CCH, 2 * H], DTM)
            nc.sync.dma_start(out=wqv[:], in_=wqv_d[:].rearrange("(nc p) h -> p nc h", p=P))
            wk = big.tile([P, CCH, H], DTM)
            nc.sync.dma_start(out=wk[:], in_=wk_d[:].rearrange("(nc p) h -> p nc h", p=P))
            ebias = big.tile([P, NBLK * SCHED[-1]], F32)
            nc.sync.dma_start(out=ebias[:], in_=eb_d[:])
            stair = big.tile([P, 896], F32)
            nc.sync.dma_start(out=stair[:], in_=st_d[:])
            ident = big.tile([P, P], DTM)
            nc.sync.dma_start(out=ident[:], in_=id_d[:])

            # ---- x loads (chunked so projections start early) ----
            xq = big.tile([P, CCH, NQ], DTM)
            xq_r = xq_d[:].rearrange("(nc p) t -> p nc t", p=P)
            for i in range(NBLK):
                nc.sync.dma_start(out=xq[:, :, i * TQ:(i + 1) * TQ],
                                  in_=xq_r[:, :, i * TQ:(i + 1) * TQ])
            xk = big.tile([P, CCH, KFULL], DTM)
            xk_r = xk_d[:].rearrange("(nc p) t -> p nc t", p=P)
            for i in range(KFULL // TQ):
                nc.sync.dma_start(out=xk[:, :, i * TQ:(i + 1) * TQ],
                                  in_=xk_r[:, :, i * TQ:(i + 1) * TQ])

            # ---- projections ----
            # qvd: rows 0:64 = qT, rows 64:128 = vdiagT   (from xq)
            qvd = big.tile([P, NQ], DTM)
            for i in range(NQ // TQ):
                ps = psp.tile([P, TQ], F32, tag="proj")
                for cc in range(CCH):
                    nc.tensor.matmul(ps[:], wqv[:, cc, :],
                                     xq[:, cc, bass.ts(i, TQ)],
                                     start=(cc == 0), stop=(cc == CCH - 1))
                nc.vector.tensor_copy(qvd[:, bass.ts(i, TQ)], ps[:])
            # kdT: [64, NQ] diag-key projection (from xq)
            kdT = big.tile([64, NQ], DTM)
            for i in range(NQ // TQ):
                ps = psp.tile([64, TQ], F32, tag="proj")
                for cc in range(CCH):
                    nc.tensor.matmul(ps[:], wk[:, cc, :],
                                     xq[:, cc, bass.ts(i, TQ)],
                                     start=(cc == 0), stop=(cc == CCH - 1))
                nc.vector.tensor_copy(kdT[:, bass.ts(i, TQ)], ps[:])
            # kv: rows 0:64 = kT, rows 64:128 = vT  (full prefix, from xk)
            kv = big.tile([P, KFULL], DTM)
            for i in range(KFULL // TQ):
                ps = psp.tile([P, TQ], F32, tag="proj")
                for cc in range(CCH):
                    nc.tensor.matmul(ps[:], wkv[:, cc, :],
                                     xk[:, cc, bass.ts(i, TQ)],
                                     start=(cc == 0), stop=(cc == CCH - 1))
                nc.vector.tensor_copy(kv[:, bass.ts(i, TQ)], ps[:])

            # ---- v_aug tiles: [128, 65] per 128-row block, col 64 = 1.0 ----
            vaug = big.tile([P, NV, H + 1], DTM)
            nc.sync.dma_start(out=vaug[:, :, H], in_=on_d[:])

            def make_vaug(slot, src_upper, col0):
                # transpose vT[64, col0:col0+128] (stored in partitions 64:128
                # of src_upper) -> vaug[:, slot, 0:64]
                tp = pstr.tile([P, H], DTM, tag="tr")
                nc.tensor.transpose(tp[:], src_upper[64:128, col0:col0 + P],
                                    ident[64:128, 64:128])
                nc.vector.tensor_copy(vaug[:, slot, 0:H], tp[:])

            for blk in range(NBLK):
                for d in range(NDIAG):
                    make_vaug(blk * NDIAG + d, qvd, blk * TQ + d * P)
            for c in range(SCHED[-1]):
                make_vaug(NBLK * NDIAG + c, kv, c * P)

            # ---- attention ----
            osb = []
            for blk in range(NBLK):
                # 96 partitions (mult of 32) so the final transpose is ISA-legal;
                # rows 65:96 are never written and their transposed cols unused.
                pv = pspv.tile([96, TQ], F32, tag="pv")
                qT = qvd[0:64, bass.ts(blk, TQ)]
                nmm = NDIAG + SCHED[blk]
                mi = 0
                # diagonal chunks (keys = own query rows, staircase mask)
                for d in range(NDIAG):
                    sp = pss.tile([P, TQ], F32, tag="s")
                    nc.tensor.matmul(sp[:], kdT[:, blk * TQ + d * P: blk * TQ + (d + 1) * P],
                                     qT, start=True, stop=True)
                    e = work.tile([P, TQ], DTM, tag="e")
                    nc.scalar.activation(e[:], sp[:], EXPF, bias=0.0, scale=SCALE)
                    off = 384 - 128 * d
                    nc.vector.tensor_mul(e[:], e[:], stair[:, off:off + TQ])
                    nc.tensor.matmul(pv[0:H + 1, :], vaug[:, blk * NDIAG + d, :],
                                     e[:], start=(mi == 0), stop=(mi == nmm - 1))
                    mi += 1
                # full chunks (bias kills acausal ones)
                for c in range(SCHED[blk]):
                    sp = pss.tile([P, TQ], F32, tag="s")
                    nc.tensor.matmul(sp[:], kv[0:64, bass.ts(c, P)],
                                     qT, start=True, stop=True)
                    e = work.tile([P, TQ], DTM, tag="e")
                    bcol = blk * SCHED[-1] + c
                    nc.scalar.activation(e[:], sp[:], EXPF,
                                         bias=ebias[:, bcol:bcol + 1], scale=SCALE)
                    nc.tensor.matmul(pv[0:H + 1, :], vaug[:, NBLK * NDIAG + c, :],
                                     e[:], start=(mi == 0), stop=(mi == nmm - 1))
                    mi += 1

                # ---- epilogue: transpose, divide, store ----
                pvs = work.tile([96, TQ], DTM, tag="pvs")
                nc.vector.tensor_copy(pvs[0:H + 1, :], pv[0:H + 1, :])
                ob = work.tile([P, TQ // P, H], F32, tag="ob")
                for j in range(TQ // P):
                    ot = pstr.tile([P, 96], DTM, tag="tr")
                    nc.tensor.transpose(ot[:], pvs[:, bass.ts(j, P)], ident[0:96, 0:96])
                    r = work.tile([P, 1], F32, tag="r")
                    nc.vector.reciprocal(r[:], ot[:, H:H + 1])
                    nc.vector.tensor_scalar_mul(ob[:, j, :], ot[:, 0:H], r[:])
                nc.sync.dma_start(
                    out=out_d[:].rearrange("(l j p) h -> p l j h", p=P, j=TQ // P)[:, blk],
                    in_=ob[:])
                osb.append(ob)
    nc.compile()
    return nc


def _host_inputs(x, Wk, Wq, Wv):
    wkv = np.ascontiguousarray(np.concatenate([Wk, Wv], axis=1), np.float32)
    wqv = np.ascontiguousarray(np.concatenate([Wq, Wv], axis=1), np.float32)
    wk = np.ascontiguousarray(Wk, np.float32)
    ii = np.arange(P)
    stair = (np.arange(896)[None, :] >= ii[:, None] + 384).astype(np.float32)
    ident = np.eye(P, dtype=np.float32)
    vones = np.ones((P, NBLK * (TQ // P) + SCHED[-1]), np.float32)
    in_maps = []
    for b in range(B):
        xT = np.ascontiguousarray(x[b].T, np.float32)  # [C, T]
        for h in range(2):
            q0s = (0, 1024) if h == 0 else (512, 1536)
            xq = np.ascontiguousarray(
                np.concatenate([xT[:, q0:q0 + TQ] for q0 in q0s], axis=1))
            xk = np.ascontiguousarray(xT[:, :KFULL])
            eb = np.zeros((P, NBLK * SCHED[-1]), np.float32)
            for blk, q0 in enumerate(q0s):
                nvalid = q0 // P  # full chunks strictly before the block
                eb[:, blk * SCHED[-1] + nvalid: blk * SCHED[-1] + SCHED[blk]] = NEG
            in_maps.append(dict(xq=xq, xk=xk, wkv=wkv, wqv=wqv, wk=wk,
                                ebias=eb, stair=stair, ident=ident, vones=vones))
    return in_maps


def kernel(x, Wk, Wq, Wv, fast_mm=True, trace=False):
    x = np.asarray(x, np.float32)
    in_maps = _host_inputs(x, np.asarray(Wk, np.float32),
                           np.asarray(Wq, np.float32), np.asarray(Wv, np.float32))
    if ("nc", fast_mm) not in _CACHE:
        _CACHE[("nc", fast_mm)] = build(fast_mm)
    nc = _CACHE[("nc", fast_mm)]
    res = run_bass_kernel_spmd(nc, in_maps, list(range(8)), trace=trace)
    out = np.empty((B, T, H), np.float32)
    for b in range(B):
        for h in range(2):
            o = res.results[b * 2 + h]["out"]
            q0s = (0, 1024) if h == 0 else (512, 1536)
            for blk, q0 in enumerate(q0s):
                out[b, q0:q0 + TQ] = o[blk * TQ:(blk + 1) * TQ]
    kernel.last_exec_time_ns = res.exec_time_ns
    kernel.last_results = res
    return out


# revision 15
# speedup vs baseline: 1.1238x; 1.1238x over previous
"""Single-head causal attention (B=4, T=2048, C=1024, H=64) on 8 NeuronCores.

Sharding: 8 cores = 4 batches x 2 interleaved halves. Core (b, h) computes
query blocks of 512 rows: h=0 -> rows [0:512] and [1024:1536]; h=1 -> rows
[512:1024] and [1536:2048]. This balances causal work while keeping ONE SPMD
program: all per-core differences enter through input DATA:
  - xq:    x[b].T columns of the core's query rows (pre-packed, bf16)
  - xk:    x[b].T columns [0:1536] (k/v prefix, pre-packed, bf16)
  - ebias: per (block, chunk) exp bias 0 / -1e30 that kills acausal chunks
           inside the ACT exp instruction (free masking).
The compile-time schedule processes max(chunks-over-cores) per block; invalid
chunks produce exp(.. - 1e30) = 0 so they contribute nothing to either the
softmax numerator or denominator.

Layout: scores are computed transposed (scoresT[tk, tq]) so softmax sums come
from the PV matmul itself: V is augmented with a ones column -> PV psum row 64
is the denominator. Diagonal (partially-causal) chunks are masked with slices
of one shared staircase tile on DVE. Matmul path is bf16 (inputs rounded on
host); accumulation and the normalize/store epilogue are fp32.

All DRAM inputs are host-prepacked to the exact SBUF layout so every big DMA
is one contiguous descriptor per partition, and DMA issue is spread over the
sync + scalar HWDGE sequencers (constants via gpsimd SWDGE) to avoid
serializing on a single engine.
"""

import numpy as np
import ml_dtypes

import concourse.bass as bass
from concourse import bacc
import concourse.mybir as mybir
import concourse.tile as tile
from concourse.bass_utils import run_bass_kernel_spmd

B, T, C, H = 4, 2048, 1024, 64
P = 128
TQ = 512                 # query block width
NBLK = 2                 # query blocks per core
NQ = NBLK * TQ           # 1024 query rows per core
SCHED = (4, 12)          # full-phase k-chunks per block (compile-time max)
NDIAG = TQ // P          # 4 diagonal chunks per block
KFULL = SCHED[-1] * P    # 1536 k columns needed for full phase
NKCH = KFULL // TQ       # 3 xk column chunks
CCH = C // P             # 8 contraction chunks
NV = NDIAG * NBLK + SCHED[-1]   # 8 diag + 12 full v blocks of 128 rows
SCALE = float(C) ** -0.5
NEG = -1e30

F32 = mybir.dt.float32
BF16 = mybir.dt.bfloat16
NPBF = ml_dtypes.bfloat16

_CACHE = {}


def build():
    nc = bacc.Bacc()
    xq_d = nc.declare_dram_parameter("xq", [NBLK, P, CCH * TQ], BF16, isOutput=False)
    xk_d = nc.declare_dram_parameter("xk", [NKCH, P, CCH * TQ], BF16, isOutput=False)
    wkv_d = nc.declare_dram_parameter("wkv", [P, CCH * 2 * H], BF16, isOutput=False)
    wqv_d = nc.declare_dram_parameter("wqv", [P, CCH * 2 * H], BF16, isOutput=False)
    wk_d = nc.declare_dram_parameter("wk", [P, CCH * H], BF16, isOutput=False)
    eb_d = nc.declare_dram_parameter("ebias", [P, NBLK * SCHED[-1]], F32, isOutput=False)
    st_d = nc.declare_dram_parameter("stair", [P, 896], BF16, isOutput=False)
    idb_d = nc.declare_dram_parameter("identb", [P, P], BF16, isOutput=False)
    idf_d = nc.declare_dram_parameter("identf", [P, P], F32, isOutput=False)
    on_d = nc.declare_dram_parameter("vones", [P, NV], BF16, isOutput=False)
    out_d = nc.declare_dram_parameter("out", [P, NBLK * NDIAG * H], F32, isOutput=True)

    EXPF = mybir.ActivationFunctionType.Exp

    with tile.TileContext(nc) as tc:
        with (
            tc.tile_pool(name="big", bufs=1) as big,
            tc.tile_pool(name="work", bufs=4) as work,
            tc.tile_pool(name="ps", bufs=2, space="PSUM") as psp,
            tc.tile_pool(name="ps_s", bufs=2, space="PSUM") as pss,
            tc.tile_pool(name="ps_pv", bufs=2, space="PSUM") as pspv,
            tc.tile_pool(name="ps_tr", bufs=1, space="PSUM") as pstr,
        ):
            # ---- DMAs: x chunks first (they gate compute), spread over the
            # two HWDGE sequencers; constants via gpsimd SWDGE ----
            hw = [nc.sync, nc.scalar]
            xqs, xks = [], []
            for i in range(NBLK):
                t = big.tile([P, CCH, TQ], BF16, tag=f"xq{i}")
                hw[i % 2].dma_start(out=t[:], in_=xq_d[i].rearrange("p (nc t) -> p nc t", nc=CCH))
                xqs.append(t)
            wqv = big.tile([P, CCH, 2 * H], BF16)
            nc.sync.dma_start(out=wqv[:], in_=wqv_d[:].rearrange("p (nc h) -> p nc h", nc=CCH))
            wkv = big.tile([P, CCH, 2 * H], BF16)
            nc.scalar.dma_start(out=wkv[:], in_=wkv_d[:].rearrange("p (nc h) -> p nc h", nc=CCH))
            wk = big.tile([P, CCH, H], BF16)
            nc.sync.dma_start(out=wk[:], in_=wk_d[:].rearrange("p (nc h) -> p nc h", nc=CCH))
            for i in range(NKCH):
                t = big.tile([P, CCH, TQ], BF16, tag=f"xk{i}")
                hw[(i + 1) % 2].dma_start(out=t[:], in_=xk_d[i].rearrange("p (nc t) -> p nc t", nc=CCH))
                xks.append(t)
            ebias = big.tile([P, NBLK * SCHED[-1]], F32)
            nc.gpsimd.dma_start(out=ebias[:], in_=eb_d[:])
            stair = big.tile([P, 896], BF16)
            nc.gpsimd.dma_start(out=stair[:], in_=st_d[:])
            identb = big.tile([P, P], BF16)
            nc.gpsimd.dma_start(out=identb[:], in_=idb_d[:])
            identf = big.tile([P, P], F32)
            nc.gpsimd.dma_start(out=identf[:], in_=idf_d[:])
            vtmp = big.tile([P, NV], BF16)
            nc.gpsimd.dma_start(out=vtmp[:], in_=on_d[:])

            # ---- projections ----
            # qvd: rows 0:64 = qT, rows 64:128 = vdiagT   (from xq)
            qvd = big.tile([P, NQ], BF16)
            for i in range(NBLK):
                ps = psp.tile([P, TQ], F32, tag="proj")
                for cc in range(CCH):
                    nc.tensor.matmul(ps[:], wqv[:, cc, :], xqs[i][:, cc, :],
                                     start=(cc == 0), stop=(cc == CCH - 1))
                nc.vector.tensor_copy(qvd[:, bass.ts(i, TQ)], ps[:])
            # kdT: [64, NQ] diag-key projection (from xq)
            kdT = big.tile([64, NQ], BF16)
            for i in range(NBLK):
                ps = psp.tile([64, TQ], F32, tag="proj")
                for cc in range(CCH):
                    nc.tensor.matmul(ps[:], wk[:, cc, :], xqs[i][:, cc, :],
                                     start=(cc == 0), stop=(cc == CCH - 1))
                nc.vector.tensor_copy(kdT[:, bass.ts(i, TQ)], ps[:])
            # kv: rows 0:64 = kT, rows 64:128 = vT  (full prefix, from xk)
            kv = big.tile([P, KFULL], BF16)
            for i in range(NKCH):
                ps = psp.tile([P, TQ], F32, tag="proj")
                for cc in range(CCH):
                    nc.tensor.matmul(ps[:], wkv[:, cc, :], xks[i][:, cc, :],
                                     start=(cc == 0), stop=(cc == CCH - 1))
                nc.vector.tensor_copy(kv[:, bass.ts(i, TQ)], ps[:])

            # ---- v_aug tiles: [128, 65] per 128-row block, col 64 = 1.0 ----
            vaug = big.tile([P, NV, H + 1], BF16)
            nc.vector.tensor_copy(vaug[:, :, H], vtmp[:])

            def make_vaug(slot, src_upper, col0):
                # transpose vT[64, col0:col0+128] (in partitions 64:128 of
                # src_upper) -> vaug[:, slot, 0:64]
                tp = pstr.tile([P, H], BF16, tag="tr")
                nc.tensor.transpose(tp[:], src_upper[64:128, col0:col0 + P],
                                    identb[64:128, 64:128])
                nc.vector.tensor_copy(vaug[:, slot, 0:H], tp[:])

            for blk in range(NBLK):
                for d in range(NDIAG):
                    make_vaug(blk * NDIAG + d, qvd, blk * TQ + d * P)
            for c in range(SCHED[-1]):
                make_vaug(NBLK * NDIAG + c, kv, c * P)

            # ---- attention ----
            for blk in range(NBLK):
                # 96 partitions (mult of 32) so the final transpose is
                # ISA-legal; rows 65:96 are never written, their cols unused.
                pv = pspv.tile([96, TQ], F32, tag="pv")
                qT = qvd[0:64, bass.ts(blk, TQ)]
                nmm = NDIAG + SCHED[blk]
                mi = 0
                # diagonal chunks (keys = own query rows, staircase mask)
                for d in range(NDIAG):
                    sp = pss.tile([P, TQ], F32, tag="s")
                    nc.tensor.matmul(sp[:], kdT[:, blk * TQ + d * P: blk * TQ + (d + 1) * P],
                                     qT, start=True, stop=True)
                    e = work.tile([P, TQ], BF16, tag="e")
                    nc.scalar.activation(e[:], sp[:], EXPF, bias=0.0, scale=SCALE)
                    off = 384 - 128 * d
                    nc.vector.tensor_mul(e[:], e[:], stair[:, off:off + TQ])
                    nc.tensor.matmul(pv[0:H + 1, :], vaug[:, blk * NDIAG + d, :],
                                     e[:], start=(mi == 0), stop=(mi == nmm - 1))
                    mi += 1
                # full chunks (bias kills acausal ones)
                for c in range(SCHED[blk]):
                    sp = pss.tile([P, TQ], F32, tag="s")
                    nc.tensor.matmul(sp[:], kv[0:64, bass.ts(c, P)], qT,
                                     start=True, stop=True)
                    e = work.tile([P, TQ], BF16, tag="e")
                    bcol = blk * SCHED[-1] + c
                    nc.scalar.activation(e[:], sp[:], EXPF,
                                         bias=ebias[:, bcol:bcol + 1], scale=SCALE)
                    nc.tensor.matmul(pv[0:H + 1, :], vaug[:, NBLK * NDIAG + c, :],
                                     e[:], start=(mi == 0), stop=(mi == nmm - 1))
                    mi += 1

                # ---- epilogue (fp32): transpose, divide, store ----
                pvs = work.tile([96, TQ], F32, tag="pvs")
                nc.vector.tensor_copy(pvs[0:H + 1, :], pv[0:H + 1, :])
                ob = work.tile([P, NDIAG, H], F32, tag="ob")
                for j in range(NDIAG):
                    ot = pstr.tile([P, 96], F32, tag="trf")
                    nc.tensor.transpose(ot[:], pvs[:, bass.ts(j, P)], identf[0:96, 0:96])
                    r = work.tile([P, 1], F32, tag="r")
                    nc.vector.reciprocal(r[:], ot[:, H:H + 1])
                    nc.vector.tensor_scalar_mul(ob[:, j, :], ot[:, 0:H], r[:])
                nc.sync.dma_start(out=out_d[:, blk * NDIAG * H:(blk + 1) * NDIAG * H],
                                  in_=ob[:])
    nc.compile()
    return nc


def _pack_x(xT, cols):
    # xT: [C, T] fp32 -> [P, CCH*W] bf16 in SBUF layout
    a = xT[:, cols]                                   # [C, W]
    a = a.reshape(CCH, P, -1).transpose(1, 0, 2)      # [P, CCH, W]
    return np.ascontiguousarray(a.reshape(P, -1)).astype(NPBF)


def _pack_w(w):
    # w: [C, width] -> [P, CCH*width]
    a = w.reshape(CCH, P, -1).transpose(1, 0, 2)
    return np.ascontiguousarray(a.reshape(P, -1)).astype(NPBF)


def _host_inputs(x, Wk, Wq, Wv):
    wkv = _pack_w(np.concatenate([Wk, Wv], axis=1))
    wqv = _pack_w(np.concatenate([Wq, Wv], axis=1))
    wk = _pack_w(Wk)
    ii = np.arange(P)
    stair = (np.arange(896)[None, :] >= ii[:, None] + 384).astype(NPBF)
    identb = np.eye(P, dtype=NPBF)
    identf = np.eye(P, dtype=np.float32)
    vones = np.ones((P, NV), NPBF)
    in_maps = []
    for b in range(B):
        xT = np.ascontiguousarray(x[b].T.astype(np.float32))  # [C, T]
        for h in range(2):
            q0s = (0, 1024) if h == 0 else (512, 1536)
            xq = np.stack([_pack_x(xT, slice(q0, q0 + TQ)) for q0 in q0s])
            xk = np.stack([_pack_x(xT, slice(i * TQ, (i + 1) * TQ))
                           for i in range(NKCH)])
            eb = np.zeros((P, NBLK * SCHED[-1]), np.float32)
            for blk, q0 in enumerate(q0s):
                nvalid = q0 // P  # full chunks strictly before the block
                eb[:, blk * SCHED[-1] + nvalid: blk * SCHED[-1] + SCHED[blk]] = NEG
            in_maps.append(dict(xq=xq, xk=xk, wkv=wkv, wqv=wqv, wk=wk,
                                ebias=eb, stair=stair, identb=identb,
                                identf=identf, vones=vones))
    return in_maps


def kernel(x, Wk, Wq, Wv, trace=False):
    x = np.asarray(x, np.float32)
    in_maps = _host_inputs(x, np.asarray(Wk, np.float32),
                           np.asarray(Wq, np.float32), np.asarray(Wv, np.float32))
    if "nc" not in _CACHE:
        _CACHE["nc"] = build()
    nc = _CACHE["nc"]
    res = run_bass_kernel_spmd(nc, in_maps, list(range(8)), trace=trace)
    out = np.empty((B, T, H), np.float32)
    for b in range(B):
        for h in range(2):
            o = res.results[b * 2 + h]["out"]  # [P, NBLK*NDIAG*H]
            o = np.asarray(o).reshape(P, NBLK, NDIAG, H)
            q0s = (0, 1024) if h == 0 else (512, 1536)
            for blk, q0 in enumerate(q0s):
                # row q0 + j*128 + p  <-  o[p, blk, j, :]
                out[b, q0:q0 + TQ] = o[:, blk].transpose(1, 0, 2).reshape(TQ, H)
    kernel.last_exec_time_ns = res.exec_time_ns
    kernel.last_results = res
    return out
